# revision 40
# baseline (speedup 1.0000x reference)
"""Multi-head attention (N=4, L=2048, E=1024, H=16, DK=64) on 8 TRN2 cores.

The reference splits heads with a PLAIN RESHAPE (n, l, H*DK) -> (n, H, l, DK),
so "head" h is really a contiguous block of 128 tokens, and the 2048 attention
positions inside it are (token, s) pairs where s indexes sixteen 64-wide
E-slices.  Per (batch, block):
    Qb = q[n, 128b:128b+128, :].reshape(2048, 64)   (same for K, V)
    out_block = softmax(Qb Kb^T / 8) Vb  -> reshape(128, E) -> rows of out
Positions are processed in permuted order p' = 128*s + tok (a permutation of
the softmax axis; unpermuted on the way out).

Sharding: core c owns token rows [n, 256c : 256c+256) for every batch n (two
128-token blocks per batch).  Outputs are disjoint rows; the host scatters.
Each core gets the full weights (streamed in halves) and only its x columns.

Matmuls in bf16 (fp32 PSUM accumulate).  Per-core flow:
  x_sb [E, 1024 tok] resident.
  V:    V_nat [128 tok, E] per (n, B), evicted into per-s slices + ones col.
  Q/K:  [e_out 128, tok 512 (2 batches)] PSUM tiles evicted straight into the
        permuted layout q1t/k1t [128 = 2B x 64 d, n, 2048 p'].
  Attention per (n, u = q' chunk of 512): 8 key-tile pairs; scores for both
        blocks row-packed on the PE (disjoint 64-row groups), written as BF16
        psum; exp on ScalarE over [128, 2048] (scale=1/8 folded), bf16 out;
        PV accumulates [V|ones].T @ expS -> [65, 512] fp32 (row 64 = denom);
        rows 0-63 evicted unnormalized into opT, denom row collected.
  Normalize per batch: one batched reciprocal [8, 512] on DVE, GPSIMD
        partition_broadcast, in-place DVE multiply on opT.
  Out proj per (n, B): accumulate over 8 e_in tiles vs woT halves, DMA out.
"""

import ml_dtypes
import numpy as np

import concourse.bass as bass
import concourse.mybir as mybir
import concourse.tile as tile
from concourse import bacc
from concourse.bass_utils import run_bass_kernel_spmd

N, L, E, H = 4, 2048, 1024, 16
DK = E // H  # 64
NC = 8
BPC = 2  # token blocks per core per batch
TPB = 128  # tokens per block
TPN = BPC * TPB  # 256 tokens per batch per core
TC = N * TPN  # 1024 tokens per core
P = 128
QC = 512  # q' chunk
NQC = 2048 // QC  # 4
NKT = 2048 // P  # 16 key tiles (= s values)
ET = E // P  # 8

F32 = mybir.dt.float32
BF16 = mybir.dt.bfloat16
MM_DT = BF16


def build_nc():
    nc = bacc.Bacc("TRN2", target_bir_lowering=False, debug=False, num_devices=NC)

    xTc = nc.dram_tensor("xTc", [E, TC], MM_DT, kind="ExternalInput").ap()
    wqT = nc.dram_tensor("wqT", [E, E], MM_DT, kind="ExternalInput").ap()
    wkT = nc.dram_tensor("wkT", [E, E], MM_DT, kind="ExternalInput").ap()
    wvT = nc.dram_tensor("wvT", [E, E], MM_DT, kind="ExternalInput").ap()
    woT = nc.dram_tensor("woT", [E, E], MM_DT, kind="ExternalInput").ap()
    outp = nc.dram_tensor("outp", [TC, E], F32, kind="ExternalOutput").ap()

    with tile.TileContext(nc) as tc:
        with (
            tc.tile_pool(name="const", bufs=1) as const,
            tc.tile_pool(name="wpool", bufs=1) as wpool,
            tc.tile_pool(name="xv", bufs=2) as xv_pool,
            tc.tile_pool(name="qk1", bufs=2) as qk1_pool,
            tc.tile_pool(name="expp", bufs=10) as exp_pool,
            tc.tile_pool(name="opt", bufs=2) as opt_pool,
            tc.tile_pool(name="nrm", bufs=2) as nrm_pool,
            tc.tile_pool(name="ops", bufs=2) as op_pool,
            tc.tile_pool(name="scps", bufs=2, space="PSUM") as sc_psum,
            tc.tile_pool(name="pvps", bufs=4, space="PSUM") as pv_psum,
        ):
            ones_f32 = const.tile([P, P], F32)
            nc.vector.memset(ones_f32[:], 1.0)
            ones_r = const.tile([P, P], mybir.dt.float32r)
            nc.vector.tensor_copy(ones_r[:], ones_f32[:])

            # ---- resident x ----
            x_sb = xv_pool.tile([P, ET, TC], MM_DT, tag="xv", name="x_sb")
            xr_ = xTc.rearrange("(a p) t -> p a t", p=P)
            for n_ in range(N):
                nc.sync.dma_start(
                    out=x_sb[:, :, n_ * TPN : (n_ + 1) * TPN],
                    in_=xr_[:, :, n_ * TPN : (n_ + 1) * TPN],
                )

            def load_w(w_dram, nm):
                w_sb = wpool.tile([P, ET, E], MM_DT, tag=nm, name=nm)
                nc.sync.dma_start(
                    out=w_sb[:], in_=w_dram.rearrange("(a p) d -> p a d", p=P)
                )
                return w_sb

            wk_sb = load_w(wkT, "wk")
            wq_sb = load_w(wqT, "wq")
            wv_sb = load_w(wvT, "wv")
            wo_sb = load_w(woT, "wo")

            def project_batch(n):
                """Returns (tiles, [chunk emitters]) so projection work for
                batch n can be interleaved into batch n-1's attention units."""
                v_sb = qk1_pool.tile(
                    [P, BPC, NKT, DK + 1], MM_DT, tag="v", name="v_sb"
                )
                q1t = qk1_pool.tile([P, 2048], MM_DT, tag="q1", name="q1t")
                k1t = qk1_pool.tile([P, 2048], MM_DT, tag="k1", name="k1t")
                groups = []
                # K and Q: [e_out 128, tok 256] psums -> permuted q1t/k1t
                def qk_group(w_sb, dst, a2):
                    def emit():
                        ps = pv_psum.tile([P, TPN], F32, tag="pv", name="qkps")
                        for a in range(ET):
                            nc.tensor.matmul(
                                ps[:],
                                w_sb[:, a, a2 * P : (a2 + 1) * P],
                                x_sb[:, a, n * TPN : (n + 1) * TPN],
                                start=(a == 0),
                                stop=(a == ET - 1),
                            )
                        for sg in range(2):
                            s = a2 * 2 + sg
                            for B in range(BPC):
                                nc.vector.tensor_copy(
                                    dst[B * DK : (B + 1) * DK,
                                        s * TPB : (s + 1) * TPB],
                                    ps[sg * DK : (sg + 1) * DK,
                                       B * TPB : (B + 1) * TPB],
                                )
                    return emit

                def v_group(B, eh):
                    def emit():
                        tok0 = n * TPN + B * TPB
                        ps = pv_psum.tile([P, 512], F32, tag="pv", name="vps")
                        for a in range(ET):
                            nc.tensor.matmul(
                                ps[:],
                                x_sb[:, a, tok0 : tok0 + TPB],
                                wv_sb[:, a, eh * 512 : (eh + 1) * 512],
                                start=(a == 0),
                                stop=(a == ET - 1),
                            )
                        nc.vector.tensor_copy(
                            v_sb[:, B, eh * 8 : (eh + 1) * 8, 0:DK],
                            ps.rearrange("p (s d) -> p s d", d=DK),
                        )
                    return emit

                def ones_group():
                    nc.vector.tensor_copy(
                        v_sb[:, :, :, DK], ones_f32[:, 0 : BPC * NKT]
                    )

                for a2 in range(ET):
                    groups.append(qk_group(wk_sb, k1t, a2))
                for a2 in range(ET):
                    groups.append(qk_group(wq_sb, q1t, a2))
                for B in range(BPC):
                    for eh in range(2):
                        groups.append(v_group(B, eh))
                groups.append(ones_group)
                return (v_sb, q1t, k1t), groups

            # ---- per batch: project, attend, normalize, out-project ----
            # `feed` holds deferred fine-grained work (next batch's projection
            # chunks, previous batch's normalize/out-proj pieces) drained one
            # item per key-tile so the PE stream never starves ScalarE.
            tiles, groups = project_batch(0)
            for g in groups:
                g()
            next_state = None
            feed = []

            def make_normalize_piece(opT, rec, B, u):
                def emit():
                    r_ = B * NQC + u
                    rp = 32 * (r_ % 4)
                    bcp = pv_psum.tile([P, QC], F32, tag="pv", name="bcp")
                    nc.tensor.matmul(
                        bcp[:],
                        ones_r[rp : rp + 1, :],
                        rec[r_ // 4][rp : rp + 1, :],
                        start=True,
                        stop=True,
                        tile_position=(rp, 0),
                    )
                    for sg in range(2):
                        tgt = opT[sg * DK : (sg + 1) * DK,
                                  2 * u : 2 * u + 2, B, :]
                        nc.vector.tensor_mul(
                            tgt,
                            tgt,
                            bcp[sg * DK : (sg + 1) * DK, :].rearrange(
                                "d (sp t) -> d sp t", t=TPB
                            )[:, sg::2, :],
                        )
                return emit

            def make_outproj_piece(opT, n, B, half):
                def emit():
                    ps = pv_psum.tile([P, 512], F32, tag="pv", name="opps")
                    for a2 in range(ET):
                        nc.tensor.matmul(
                            ps[:],
                            opT[:, a2, B, :],
                            wo_sb[:, a2, half * 512 : (half + 1) * 512],
                            start=(a2 == 0),
                            stop=(a2 == ET - 1),
                        )
                    op_sb = op_pool.tile([P, 512], F32, tag="op")
                    nc.vector.tensor_copy(op_sb[:], ps[:])
                    r0 = n * TPN + B * TPB
                    nc.sync.dma_start(
                        out=outp[r0 : r0 + TPB, half * 512 : (half + 1) * 512],
                        in_=op_sb[:],
                    )
                return emit

            for n in range(N):
                while feed:
                    feed.pop(0)()  # safety drain before slot-reusing allocs
                v_sb, q1t, k1t = tiles
                if n + 1 < N:
                    next_state = project_batch(n + 1)
                    feed.extend(next_state[1])
                opT = opt_pool.tile([P, ET, BPC, TPB], MM_DT, tag="opT", name="opT")
                # denominator rows live at 32-aligned partitions of two tiles
                sums = [
                    nrm_pool.tile([P, QC], F32, tag="sums", name=f"sums{_i}")
                    for _i in range(2)
                ]
                for u in range(NQC):
                    # drain deferred work in small lumps at unit boundaries
                    take = (len(feed) + NQC - 1 - u) // (NQC - u) if feed else 0
                    for _ in range(min(take, len(feed))):
                        feed.pop(0)()
                    qsl = slice(u * QC, (u + 1) * QC)
                    pv = [
                        pv_psum.tile([DK + 1, QC], F32, tag="pv", name=f"pv{_b}")
                        for _b in range(BPC)
                    ]
                    for j in range(NKT):
                        sc = sc_psum.tile([P, BPC, QC], F32, tag="sc")
                        ksl = slice(j * TPB, (j + 1) * TPB)
                        for B in range(BPC):
                            bsl = slice(B * DK, (B + 1) * DK)
                            nc.tensor.matmul(
                                sc[:, B, :],
                                k1t[bsl, ksl],
                                q1t[bsl, qsl],
                                start=True,
                                stop=True,
                            )
                        exps = exp_pool.tile([P, BPC, QC], MM_DT, tag="exps")
                        nc.scalar.activation(
                            exps[:],
                            sc[:],
                            mybir.ActivationFunctionType.Exp,
                            scale=1.0 / np.sqrt(DK),
                        )
                        for B in range(BPC):
                            nc.tensor.matmul(
                                pv[B][:],
                                v_sb[:, B, j, :],
                                exps[:, B, :],
                                start=(j == 0),
                                stop=(j == NKT - 1),
                            )
                    for B in range(BPC):
                        # unnormalized eviction into opT; s = 4u + sp
                        for sg in range(2):
                            nc.vector.tensor_copy(
                                opT[sg * DK : (sg + 1) * DK,
                                    2 * u : 2 * u + 2, B, :],
                                pv[B][0:DK, :].rearrange(
                                    "d (sp t) -> d sp t", t=TPB
                                )[:, sg::2, :],
                            )
                        r_ = B * NQC + u
                        nc.vector.tensor_copy(
                            sums[r_ // 4][32 * (r_ % 4) : 32 * (r_ % 4) + 1, :],
                            pv[B][DK : DK + 1, :],
                        )

                rec = [
                    nrm_pool.tile([P, QC], mybir.dt.float32r, tag="rec",
                                  name=f"rec{_i}")
                    for _i in range(2)
                ]
                with nc.allow_low_precision(reason="softmax denominators"):
                    for _i in range(2):
                        nc.vector.reciprocal(rec[_i][:], sums[_i][:])
                for B in range(BPC):
                    for u in range(NQC):
                        feed.append(make_normalize_piece(opT, rec, B, u))
                for B in range(BPC):
                    for half in range(2):
                        feed.append(make_outproj_piece(opT, n, B, half))
                if next_state is not None:
                    tiles = next_state[0]

            while feed:
                feed.pop(0)()

    nc.compile()
    return nc


_CACHED_NC = None


def get_nc():
    global _CACHED_NC
    if _CACHED_NC is None:
        _CACHED_NC = build_nc()
    return _CACHED_NC


def make_in_maps(inputs):
    x = np.ascontiguousarray(np.asarray(inputs["x"], dtype=np.float32))
    Wq = np.asarray(inputs["Wq"], dtype=np.float32)
    Wk = np.asarray(inputs["Wk"], dtype=np.float32)
    Wv = np.asarray(inputs["Wv"], dtype=np.float32)
    Wo = np.asarray(inputs["Wo"], dtype=np.float32)

    def cast(a):
        return np.ascontiguousarray(a).astype(ml_dtypes.bfloat16)

    wqT = cast(Wq.T)
    wkT = cast(Wk.T)
    wvT = cast(Wv.T)
    woT = cast(Wo.T)
    xr = x.reshape(N, L, E)

    in_maps = []
    for c in range(NC):
        xc = np.concatenate(
            [xr[n, 256 * c : 256 * (c + 1), :] for n in range(N)], axis=0
        )
        in_maps.append(
            {
                "xTc": cast(xc.T),
                "wqT": wqT,
                "wkT": wkT,
                "wvT": wvT,
                "woT": woT,
            }
        )
    return in_maps


def kernel(x, Wq, Wk, Wv, Wo):
    in_maps = make_in_maps({"x": x, "Wq": Wq, "Wk": Wk, "Wv": Wv, "Wo": Wo})
    res = run_bass_kernel_spmd(get_nc(), in_maps, list(range(NC)))
    out = np.empty((N, L, E), dtype=np.float32)
    for c in range(NC):
        o = res.results[c]["outp"].reshape(N, TPN, E)
        out[:, 256 * c : 256 * (c + 1), :] = o
    return out
